# revision 10
# baseline (speedup 1.0000x reference)
"""Trainium2 Bass kernel for nn_NeuralGeodesicFlows.

Math (the reference's Christoffel t1/t3 cancel since g is symmetric):
    u = x @ W1 + b1 ; h = tanh(u) ; t = 1 - h^2
    Afl = h @ W2 + b2 ; A = Afl.reshape(8, 8) ; G = A A^T + I
    c = A^T v ; vc = vec(v c^T) ; y = W2 @ vc ; s = W1 @ (t * y)
    dv = -G^{-1} s ; dx = v
    RK4 with num_steps steps, dt = t/num_steps.

Layouts per 128-sample block (nb blocks per core, 8 cores):
    sample-partition (s-p): [128 samples, per-sample free dims]
    feature-partition (f-p): [feature, samples]
PE matmuls alternate layouts (H / TY serve as transposed stationary
operands), with explicit PE transposes only for x and vc. The SPD solve
is CG with per-sample scalars via free-dim broadcast APs. PE inputs and
the CG matvec products run in bf16 (validated: output rel err ~2.5e-4).
"""
import numpy as np
from contextlib import ExitStack

try:
    import concourse.bass as bass
except ImportError:
    import sys
    for _p in ("/opt/trn_rl_repo", "/root/.axon_site/_ro/trn_rl_repo"):
        if _p not in sys.path:
            sys.path.insert(0, _p)
    import concourse.bass as bass

import concourse.tile as tile
from concourse import bacc, mybir

F32 = mybir.dt.float32
BF16 = mybir.dt.bfloat16
AX = mybir.AxisListType
OP = mybir.AluOpType
AF = mybir.ActivationFunctionType

N_CORES = 8
M = 8
HID = 64


def _ins(apv, idx, count):
    """Insert a broadcast (step-0) dim into an AP at position idx
    (0 = partition dim)."""
    dims = [list(d) for d in apv.ap]
    dims.insert(idx, [0, count])
    return bass.AP(tensor=apv.tensor, offset=apv.offset, ap=dims)


def build_program(nb, nsteps, dt, cg_iters=3):
    """Build the per-core Bass program: nb 128-sample blocks, RK4 nsteps."""
    P = 128
    S = nb * P
    XG = 4 if nb % 4 == 0 else 1     # blocks per packed x-transpose
    VG = 2 if nb % 2 == 0 else 1     # blocks per packed vc-transpose

    nc = bacc.Bacc()
    zin = nc.declare_dram_parameter("z_in", [S, 16], F32, isOutput=False)
    w1d = nc.declare_dram_parameter("w1", [M, HID], BF16, isOutput=False)
    w2d = nc.declare_dram_parameter("w2", [HID, HID], BF16, isOutput=False)
    w2td = nc.declare_dram_parameter("w2t", [HID, HID], BF16, isOutput=False)
    w1td = nc.declare_dram_parameter("w1t", [HID, M], BF16, isOutput=False)
    b1d = nc.declare_dram_parameter("b1", [HID], F32, isOutput=False)
    b2d = nc.declare_dram_parameter("b2", [HID], F32, isOutput=False)
    idnd = nc.declare_dram_parameter("idn", [P, P], BF16, isOutput=False)
    zoutd = nc.declare_dram_parameter("z_out", [S, 16], F32, isOutput=True)

    with ExitStack() as ctx:
        tc = ctx.enter_context(tile.TileContext(nc))
        const = ctx.enter_context(tc.tile_pool(name="const", bufs=1))
        statep = ctx.enter_context(tc.tile_pool(name="state", bufs=1))
        big = ctx.enter_context(tc.tile_pool(name="big", bufs=2))
        scr = ctx.enter_context(tc.tile_pool(name="scr", bufs=2))
        cgp = ctx.enter_context(tc.tile_pool(name="cg", bufs=2))
        psq = ctx.enter_context(tc.tile_pool(name="psq", bufs=5, space="PSUM"))
        psb = ctx.enter_context(tc.tile_pool(name="psb", bufs=1, space="PSUM"))

        # ---- constants to SBUF (single SWDGE queue: one wait sem) ----
        w1s = const.tile([M, HID], BF16)
        nc.gpsimd.dma_start(out=w1s[:], in_=w1d[:, :])
        w2s = const.tile([HID, HID], BF16)
        nc.gpsimd.dma_start(out=w2s[:], in_=w2d[:, :])
        w2ts = const.tile([HID, HID], BF16)
        nc.gpsimd.dma_start(out=w2ts[:], in_=w2td[:, :])
        w1ts = const.tile([HID, M], BF16)
        nc.gpsimd.dma_start(out=w1ts[:], in_=w1td[:, :])
        idns = const.tile([P, P], BF16)
        nc.gpsimd.dma_start(out=idns[:], in_=idnd[:, :])
        b1s = const.tile([HID, 1], F32)
        b1ap = b1d[:]
        nc.gpsimd.dma_start(
            out=b1s[:],
            in_=bass.AP(tensor=b1ap.tensor, offset=b1ap.offset,
                        ap=[list(b1ap.ap[0]), [0, 1]]),
        )
        b2s = const.tile([P, HID], F32)   # b2 replicated on all partitions
        b2ap = b2d[:]
        nc.gpsimd.dma_start(
            out=b2s[:],
            in_=bass.AP(tensor=b2ap.tensor, offset=b2ap.offset,
                        ap=[[0, P], list(b2ap.ap[0])]),
        )

        # ---- state ----
        z = statep.tile([P, nb * 16], F32)
        acc = statep.tile([P, nb * 16], F32)
        z_v3 = z[:].rearrange("p (b i) -> p b i", i=16)
        nc.gpsimd.dma_start(out=z_v3, in_=zin[:, :].rearrange("(b p) i -> p b i", p=P))

        def rhs(src, kt):
            """Emit one RHS eval: src [P, nb*16] state tile -> kt [P, nb*16]."""
            srcv = src[:].rearrange("p (b i) -> p b i", i=16)
            vv = srcv[:, :, 8:16]                      # [p, b, r] fp32

            # packed bf16 casts of x and v halves
            zxb = scr.tile([P, nb * M], BF16, tag="zxb")
            nc.vector.tensor_copy(
                out=zxb[:].rearrange("p (b i) -> p b i", i=M),
                in_=srcv[:, :, 0:8])
            vxb = scr.tile([P, nb * M], BF16, tag="vxb")
            nc.vector.tensor_copy(
                out=vxb[:].rearrange("p (b i) -> p b i", i=M),
                in_=vv)

            Hs = big.tile([HID, nb * P], BF16, tag="H")
            Ts = big.tile([HID, nb * P], F32, tag="T")

            # metric MLP layer 1 (f-p): packed x-transpose, U = W1^T X^T, tanh
            for g in range(nb // XG):
                xt_ps = psq.tile([M * XG, P], BF16, tag="q")
                nc.tensor.transpose(xt_ps[:],
                                    zxb[:, M * XG * g:M * XG * (g + 1)], idns[:])
                xf = scr.tile([M * XG, P], BF16, tag="xf")
                nc.scalar.activation(out=xf[:], in_=xt_ps[:], func=AF.Copy)
                u_ps = psq.tile([HID, XG * P], F32, tag="q")
                for j in range(XG):
                    nc.tensor.matmul(u_ps[:, P * j:P * (j + 1)], lhsT=w1s[:],
                                     rhs=xf[M * j:M * (j + 1), :],
                                     start=True, stop=True)
                sl = slice(g * XG * P, (g + 1) * XG * P)
                nc.scalar.activation(out=Hs[:, sl], in_=u_ps[:],
                                     func=AF.Tanh, bias=b1s[:], scale=1.0)
                hsq = scr.tile([HID, XG * P], F32, tag="hsq")
                nc.scalar.activation(out=hsq[:], in_=Hs[:, sl], func=AF.Square)
                # T = (hsq - 1) * -1 = 1 - h^2
                nc.vector.tensor_scalar(out=Ts[:, sl], in0=hsq[:],
                                        scalar1=1.0, scalar2=-1.0,
                                        op0=OP.subtract, op1=OP.mult)

            # metric MLP layer 2 (s-p): Afl = H^T-block @ W2 + b2
            afl_ps = psb.tile([P, nb * HID], F32, tag="afl")
            for b in range(nb):
                nc.tensor.matmul(afl_ps[:, HID * b:HID * (b + 1)],
                                 lhsT=Hs[:, P * b:P * (b + 1)], rhs=w2s[:],
                                 start=True, stop=True)
            # two bf16 copies of Afl: cr-packed (pos 8c+r) and rc-packed (8r+c)
            acr = big.tile([P, nb * HID], BF16, tag="acr")
            acr_pk = acr[:].rearrange("p (b c r) -> p b c r", c=M, r=M)
            nc.vector.tensor_tensor(
                out=acr_pk,
                in0=afl_ps[:].rearrange("p (b r c) -> p b c r", r=M, c=M),
                in1=bass.AP(tensor=b2s[:].tensor, offset=b2s[:].offset,
                            ap=[list(b2s[:].ap[0]), [0, nb], [1, M], [M, M]]),
                op=OP.add)
            arc = big.tile([P, nb * HID], BF16, tag="arc")
            nc.gpsimd.tensor_copy(
                out=arc[:].rearrange("p (b r c) -> p b r c", r=M, c=M),
                in_=acr[:].rearrange("p (b c r) -> p b r c", c=M, r=M))
            arc_pk = arc[:].rearrange("p (b r c) -> p b r c", r=M, c=M)

            # c = A^T v  (s-p, all-packed bf16 mul at 2x)
            p1 = scr.tile([P, nb * HID], BF16, tag="p1")
            p1_v = p1[:].rearrange("p (b c r) -> p b c r", c=M, r=M)
            nc.vector.tensor_tensor(
                out=p1_v, in0=acr_pk,
                in1=_ins(vxb[:].rearrange("p (b r) -> p b r", r=M), 2, M),
                op=OP.mult)
            csp = cgp.tile([P, nb * M], F32, tag="c")
            csp_v = csp[:].rearrange("p (b c) -> p b c", c=M)
            nc.vector.reduce_sum(out=csp_v, in_=p1_v, axis=AX.X)

            # vc = vec(v c^T)  (s-p, bf16 out, rc-packed to match W2 flatten)
            vc = scr.tile([P, nb * HID], BF16, tag="vc")
            nc.vector.tensor_tensor(
                out=vc[:].rearrange("p (b r c) -> p b r c", r=M, c=M),
                in0=_ins(vv, 3, M), in1=_ins(csp_v, 2, M), op=OP.mult)

            # y (f-p): packed vc-transpose, Y = W2T^T VC^T;  TY = T * Y
            TY = big.tile([HID, nb * P], BF16, tag="TY")
            for g in range(nb // VG):
                vt_ps = psq.tile([HID * VG, P], BF16, tag="q")
                nc.tensor.transpose(vt_ps[:],
                                    vc[:, HID * VG * g:HID * VG * (g + 1)],
                                    idns[:])
                vf = scr.tile([HID * VG, P], BF16, tag="vf")
                if g % 2 == 0:
                    nc.scalar.activation(out=vf[:], in_=vt_ps[:], func=AF.Copy)
                else:
                    nc.vector.tensor_copy(out=vf[:], in_=vt_ps[:])
                y_ps = psq.tile([HID, VG * P], F32, tag="q")
                for j in range(VG):
                    b = g * VG + j
                    nc.tensor.matmul(y_ps[:, P * j:P * (j + 1)], lhsT=w2ts[:],
                                     rhs=vf[HID * j:HID * (j + 1), :],
                                     start=True, stop=True)
                sl = slice(g * VG * P, (g + 1) * VG * P)
                nc.vector.tensor_tensor(out=TY[:, sl], in0=Ts[:, sl],
                                        in1=y_ps[:], op=OP.mult)

            # s (s-p): s = TY^T-block @ W1T
            s_ps = psq.tile([P, nb * M], F32, tag="q")
            for b in range(nb):
                nc.tensor.matmul(s_ps[:, M * b:M * (b + 1)],
                                 lhsT=TY[:, P * b:P * (b + 1)], rhs=w1ts[:],
                                 start=True, stop=True)

            # ---- CG solve G w = s with G = A A^T + I  (all s-p) ----
            res = cgp.tile([P, nb * M], F32, tag="res")
            nc.vector.tensor_copy(out=res[:], in_=s_ps[:])
            pt = cgp.tile([P, nb * M], F32, tag="pt")
            nc.scalar.activation(out=pt[:], in_=s_ps[:], func=AF.Copy)
            ptb = cgp.tile([P, nb * M], BF16, tag="ptb")
            nc.gpsimd.tensor_copy(out=ptb[:], in_=res[:])
            xt = cgp.tile([P, nb * M], F32, tag="xt")
            nc.gpsimd.memset(xt[:], 0.0)
            mtmp = cgp.tile([P, nb * M], F32, tag="mtmp")
            rr = cgp.tile([P, nb], F32, tag="rr")
            nc.vector.tensor_mul(mtmp[:], res[:], res[:])
            nc.vector.reduce_sum(out=rr[:],
                                 in_=mtmp[:].rearrange("p (b i) -> p b i", i=M),
                                 axis=AX.X)
            nc.vector.tensor_scalar_add(rr[:], rr[:], 1e-30)

            for it in range(cg_iters):
                pt_v = pt[:].rearrange("p (b i) -> p b i", i=M)
                ptb_v = ptb[:].rearrange("p (b i) -> p b i", i=M)
                # Gp = A (A^T p) + p  (bf16 packed muls at 2x)
                p2 = scr.tile([P, nb * HID], BF16, tag="p1")
                p2_v = p2[:].rearrange("p (b c r) -> p b c r", c=M, r=M)
                nc.vector.tensor_tensor(out=p2_v, in0=acr_pk,
                                        in1=_ins(ptb_v, 2, M), op=OP.mult)
                wtb = cgp.tile([P, nb * M], BF16, tag="wtb")
                wtb_v = wtb[:].rearrange("p (b c) -> p b c", c=M)
                with nc.allow_low_precision("cg matvec intermediate"):
                    nc.vector.reduce_sum(out=wtb_v, in_=p2_v, axis=AX.X)
                p3 = scr.tile([P, nb * HID], BF16, tag="p1")
                p3_v = p3[:].rearrange("p (b r c) -> p b r c", r=M, c=M)
                nc.vector.tensor_tensor(out=p3_v, in0=arc_pk,
                                        in1=_ins(wtb_v, 2, M), op=OP.mult)
                gp = cgp.tile([P, nb * M], F32, tag="gp")
                gp_v = gp[:].rearrange("p (b r) -> p b r", r=M)
                nc.vector.reduce_sum(out=gp_v, in_=p3_v, axis=AX.X)
                nc.vector.tensor_add(gp[:], gp[:], pt[:])
                # alpha = rr / (p.Gp + eps)
                nc.vector.tensor_mul(mtmp[:], pt[:], gp[:])
                pap = cgp.tile([P, nb], F32, tag="pap")
                nc.vector.reduce_sum(out=pap[:],
                                     in_=mtmp[:].rearrange("p (b i) -> p b i", i=M),
                                     axis=AX.X)
                nc.vector.tensor_scalar_add(pap[:], pap[:], 1e-30)
                nc.vector.reciprocal(out=pap[:], in_=pap[:])
                al = cgp.tile([P, nb], F32, tag="al")
                nc.gpsimd.tensor_mul(al[:], rr[:], pap[:])
                # x += alpha * p   (gpsimd, parallel with DVE residual path)
                t1 = cgp.tile([P, nb * M], F32, tag="t1")
                t1_v = t1[:].rearrange("p (b i) -> p b i", i=M)
                nc.gpsimd.tensor_tensor(out=t1_v, in0=_ins(al[:], 2, M),
                                        in1=pt_v, op=OP.mult)
                nc.gpsimd.tensor_add(xt[:], xt[:], t1[:])
                if it == cg_iters - 1:
                    break
                # res -= alpha * Gp
                t2 = cgp.tile([P, nb * M], F32, tag="t2")
                t2_v = t2[:].rearrange("p (b i) -> p b i", i=M)
                nc.vector.tensor_tensor(out=t2_v, in0=_ins(al[:], 2, M),
                                        in1=gp_v, op=OP.mult)
                nc.vector.tensor_sub(res[:], res[:], t2[:])
                # beta = rr_new / rr ; p = res + beta * p
                nc.vector.tensor_mul(mtmp[:], res[:], res[:])
                rr2 = cgp.tile([P, nb], F32, tag="rr2")
                nc.vector.reduce_sum(out=rr2[:],
                                     in_=mtmp[:].rearrange("p (b i) -> p b i", i=M),
                                     axis=AX.X)
                nc.vector.tensor_scalar_add(rr2[:], rr2[:], 1e-30)
                rrinv = cgp.tile([P, nb], F32, tag="rrinv")
                nc.vector.reciprocal(out=rrinv[:], in_=rr[:])
                be = cgp.tile([P, nb], F32, tag="al")
                nc.gpsimd.tensor_mul(be[:], rr2[:], rrinv[:])
                t3 = cgp.tile([P, nb * M], F32, tag="t1")
                t3_v = t3[:].rearrange("p (b i) -> p b i", i=M)
                nc.gpsimd.tensor_tensor(out=t3_v, in0=_ins(be[:], 2, M),
                                        in1=pt_v, op=OP.mult)
                nc.gpsimd.tensor_add(pt[:], res[:], t3[:])
                nc.gpsimd.tensor_copy(out=ptb[:], in_=pt[:])
                rr = rr2

            # k = [v, -x]
            ktv = kt[:].rearrange("p (b i) -> p b i", i=16)
            nc.gpsimd.tensor_copy(out=ktv[:, :, 0:8], in_=vv)
            nc.gpsimd.tensor_scalar_mul(
                out=ktv[:, :, 8:16],
                in0=xt[:].rearrange("p (b i) -> p b i", i=M), scalar1=-1.0)

        def stt(out_t, k_t, a, in_t):
            nc.vector.scalar_tensor_tensor(out=out_t[:], in0=k_t[:], scalar=a,
                                           in1=in_t[:], op0=OP.mult, op1=OP.add)

        for step in range(nsteps):
            k1 = big.tile([P, nb * 16], F32, tag="k")
            rhs(z, k1)
            stt(acc, k1, dt / 6.0, z)
            zt1 = big.tile([P, nb * 16], F32, tag="zt")
            stt(zt1, k1, dt / 2.0, z)
            k2 = big.tile([P, nb * 16], F32, tag="k")
            rhs(zt1, k2)
            stt(acc, k2, dt / 3.0, acc)
            zt2 = big.tile([P, nb * 16], F32, tag="zt")
            stt(zt2, k2, dt / 2.0, z)
            k3 = big.tile([P, nb * 16], F32, tag="k")
            rhs(zt2, k3)
            stt(acc, k3, dt / 3.0, acc)
            zt3 = big.tile([P, nb * 16], F32, tag="zt")
            stt(zt3, k3, dt / 1.0, z)
            k4 = big.tile([P, nb * 16], F32, tag="k")
            rhs(zt3, k4)
            stt(z, k4, dt / 6.0, acc)

        nc.sync.dma_start(out=zoutd[:, :].rearrange("(b p) i -> p b i", p=P),
                          in_=z_v3)
    nc.finalize()
    return nc


_CACHE = {}


def _get_program(nb, nsteps, dt, cg_iters=3):
    key = (nb, nsteps, float(dt), cg_iters)
    if key not in _CACHE:
        _CACHE[key] = build_program(nb, nsteps, float(dt), cg_iters)
    return _CACHE[key]


def kernel(z, W1, b1, W2, b2, t, num_steps):
    import ml_dtypes
    from concourse.bass_utils import run_bass_kernel_spmd

    z = np.ascontiguousarray(np.asarray(z, dtype=np.float32))
    W1 = np.asarray(W1, dtype=np.float32)
    b1 = np.asarray(b1, dtype=np.float32)
    W2 = np.asarray(W2, dtype=np.float32)
    b2 = np.asarray(b2, dtype=np.float32)
    nsteps = int(np.asarray(num_steps))
    tval = float(np.asarray(t).reshape(-1)[0])

    B = z.shape[0]
    if nsteps == 0:
        return z.copy()
    dt = np.float32(tval) / np.float32(nsteps)
    assert B % (N_CORES * 128) == 0, f"batch {B} not divisible by {N_CORES * 128}"
    S = B // N_CORES
    nb = S // 128

    nc = _get_program(nb, nsteps, float(dt))

    bf = ml_dtypes.bfloat16
    common = {
        "w1": W1.astype(bf), "w2": W2.astype(bf),
        "w2t": np.ascontiguousarray(W2.T).astype(bf),
        "w1t": np.ascontiguousarray(W1.T).astype(bf),
        "b1": b1, "b2": b2,
        "idn": np.eye(128, dtype=np.float32).astype(bf),
    }
    in_maps = [dict(common, z_in=z[i * S:(i + 1) * S]) for i in range(N_CORES)]
    out = run_bass_kernel_spmd(nc, in_maps, list(range(N_CORES)))
    return np.concatenate([out.results[i]["z_out"] for i in range(N_CORES)], axis=0)
